# revision 13
# baseline (speedup 1.0000x reference)
"""Binary 3-layer CNN (sign activations + sign weights) on 8 NeuronCores.

Strategy: pure data parallel — 64 images -> 8 cores x 8 images.
Per core: 2 batches of 4 images; SBUF partition layout [128 = (4 img, 32 ch)].

This substrate has a large fixed cost per *instruction* (~50-100 us) that is
nearly independent of instruction size, so the design minimizes instruction
count:
 - conv0: all 9 taps packed into the contraction dim (K=36 = 9 taps x 4
   images, via 9 tap-shifted DMA loads) -> ONE matmul per PSUM fill.
 - conv1/conv2: operands in fp8e4 (values +-1/0, exact) so tap-PAIRS pack
   into one DoubleRow matmul (2 K-tiles per instruction) -> 5 matmuls per
   fill instead of 9. rhs k-tile dim is a hand-built access pattern whose
   stride is the inter-tap offset (dy*WP + dx delta). DoubleRow fp8 weight
   loads require M=128, so conv2's 4-wide weights are zero-padded to 128.
 - Matmul PSUM output is ISA-capped at one 512-f32 bank; one 8-bank pool
   tile [128, 4096] takes 8 matmul bank-slices and is evicted by ONE Sign
   activation.
All matmul operands are exactly +-1/0 -> fp32 PSUM accumulation is exact.
Layers staged through padded DRAM scratch so conv padding is baked in.
(An all-f32 variant avoids the per-matmul InstLdweights split but measures
slower: f32 matmul slots cost ~80us vs ~54us for fp8 -- fp8+DoubleRow wins.)
"""

import numpy as np
import ml_dtypes

import concourse.mybir as mybir
import concourse.tile as tile
from concourse import bacc
from concourse.bass_types import AP as RawAP
from concourse.bass_utils import run_bass_kernel_spmd

BF16 = mybir.dt.bfloat16
FP8 = mybir.dt.float8e4
F32 = mybir.dt.float32
F16 = mybir.dt.float16
AF = mybir.ActivationFunctionType
DR = mybir.MatmulPerfMode.DoubleRow

N_CORES = 8
IMG_PER_CORE = 8
B = 4          # images per partition-batch
H = W = 256
WP = 258       # padded width
HP = 258       # padded height
R = 64         # strip rows (B/C)
RA = 128       # strip rows (A)
NB = IMG_PER_CORE // B
TAPS = [(t // 3, t % 3) for t in range(9)]


def _conv_taps_dr(nc, ps_slice, s_t, hin, r):
    """9-tap conv into one 512-f32 psum bank slice: 4 DoubleRow + 1 single.
    hin is a padded fp8 tile [*, rows, WP]; r is the strip-local out row."""
    for pi, t0 in enumerate((0, 2, 4, 6)):
        dy0, dx0 = TAPS[t0]
        dy1, dx1 = TAPS[t0 + 1]
        delta = (dy1 - dy0) * WP + (dx1 - dx0)
        base = hin[:, r + dy0:r + dy0 + 2, dx0:dx0 + 256]
        rhs = RawAP(tensor=base.tensor,
                    ap=[[base.ap[0][0], 128], [delta, 2], [WP, 2], [1, 256]],
                    offset=base.offset)
        nc.tensor.matmul(ps_slice, s_t[:, t0:t0 + 2, :], rhs,
                         start=(pi == 0), stop=False, perf_mode=DR)
    nc.tensor.matmul(ps_slice, s_t[:, 8, :], hin[:, r + 2:r + 4, 2:WP],
                     start=False, stop=True)


def _build_program(stages=('0', 'A', 'B', 'C')):
    nc = bacc.Bacc("TRN2", target_bir_lowering=False, debug=False)

    x_in = nc.dram_tensor("x", [IMG_PER_CORE, H, W], F32, kind="ExternalInput")
    s0_in = nc.dram_tensor("s0", [36, 128], FP8, kind="ExternalInput")
    s1_in = nc.dram_tensor("s1", [128, 9, 128], FP8, kind="ExternalInput")
    s2_in = nc.dram_tensor("s2", [128, 9, 128], FP8, kind="ExternalInput")
    out_d = nc.dram_tensor("out", [IMG_PER_CORE, H, W], F16, kind="ExternalOutput")

    xs_d = nc.dram_tensor("xs", [IMG_PER_CORE, HP, WP], FP8)
    h0_d = nc.dram_tensor("h0", [NB, 128, HP, WP], FP8)
    h1_d = nc.dram_tensor("h1", [NB, 128, HP, WP], FP8)

    with tile.TileContext(nc) as tc:
        with (
            tc.tile_pool(name="const", bufs=1) as cpool,
            tc.tile_pool(name="xprep", bufs=2) as xpool,
            tc.tile_pool(name="a_in", bufs=1) as a_in_pool,
            tc.tile_pool(name="lay_in", bufs=2) as inpool,
            tc.tile_pool(name="lay_out", bufs=2) as outpool,
            tc.tile_pool(name="c_out", bufs=1) as cpool2,
            tc.tile_pool(name="psum", bufs=1, space="PSUM") as pspool,
        ):
            # --- constants ---
            s0t = cpool.tile([36, 128], FP8, tag="s0")
            nc.sync.dma_start(out=s0t[:, :], in_=s0_in[:, :])
            s1t = cpool.tile([128, 9, 128], FP8, tag="s1")
            nc.sync.dma_start(out=s1t[:, :, :], in_=s1_in[:, :, :])
            s2t = cpool.tile([128, 9, 128], FP8, tag="s2")
            nc.sync.dma_start(out=s2t[:, :, :], in_=s2_in[:, :, :])
            zt = cpool.tile([128, WP], FP8, tag="zt")
            nc.gpsimd.memset(zt[:, :], 0.0)
            ztw = cpool.tile([1, IMG_PER_CORE * WP], FP8, tag="ztw")
            nc.gpsimd.memset(ztw[:, :], 0.0)

            # --- pre-zero DRAM pad rows ---
            for img in range(IMG_PER_CORE):
                nc.scalar.dma_start(out=xs_d[img, 0:1, :], in_=ztw[:, 0:WP])
                nc.scalar.dma_start(out=xs_d[img, HP - 1:HP, :],
                                    in_=ztw[:, 0:WP])
            for b in range(NB):
                for hd in (h0_d, h1_d):
                    nc.scalar.dma_start(out=hd[b, :, 0, :], in_=zt[:, :])
                    nc.scalar.dma_start(out=hd[b, :, HP - 1, :], in_=zt[:, :])

            # --- stage 0: sign(x) -> padded fp8 planes in DRAM, 4 img/tile ---
            if '0' in stages:
                for b in range(NB):
                    for rb in range(H // 128):
                        xf = xpool.tile([128, B, W], F32, tag="xf")
                        src = x_in[b * B:(b + 1) * B,
                                   rb * 128:(rb + 1) * 128, :]
                        nc.sync.dma_start(out=xf[:, :, :],
                                          in_=src.transpose([1, 0, 2]))
                        xp = xpool.tile([128, B, WP], FP8, tag="xp")
                        nc.vector.memset(xp[:, :, :], 0.0)
                        nc.scalar.activation(xp[:, :, 1:W + 1], xf[:, :, :],
                                             AF.Sign)
                        dst = xs_d[b * B:(b + 1) * B,
                                   rb * 128 + 1:(rb + 1) * 128 + 1, :]
                        nc.scalar.dma_start(out=dst.transpose([1, 0, 2]),
                                            in_=xp[:, :, :])

            for b in range(NB):
                # ---- stage A: conv0 (1 -> 32ch); K=36 (9 taps x 4 img) ----
                # x36[(t,g), r, c] = xs[g, RA*s + r + dy_t, dx_t + c]
                if 'A' in stages:
                    for s in range(H // RA):
                        x36 = a_in_pool.tile([36, RA, W], FP8, tag="x36")
                        for t, (dy, dx) in enumerate(TAPS):
                            nc.sync.dma_start(
                                out=x36[B * t:B * (t + 1), :, :],
                                in_=xs_d[b * B:(b + 1) * B,
                                         s * RA + dy:s * RA + dy + RA,
                                         dx:dx + 256])
                        ht = outpool.tile([128, RA, WP], FP8, tag="hta",
                                          bufs=1)
                        nc.vector.memset(ht[:, :, :], 0.0)
                        for i in range(RA // 16):
                            ps = pspool.tile([128, 4096], F32, tag="ps",
                                             name="ps")
                            for fi in range(8):
                                r = 16 * i + 2 * fi
                                nc.tensor.matmul(
                                    ps[:, fi * 512:(fi + 1) * 512],
                                    s0t[:, :], x36[:, r:r + 2, :],
                                    start=True, stop=True)
                            nc.scalar.activation(
                                ht[:, 16 * i:16 * i + 16, 1:W + 1],
                                ps[:, :].rearrange("p (a c) -> p a c", a=16),
                                AF.Sign)
                        nc.scalar.dma_start(
                            out=h0_d[b, :, s * RA + 1:s * RA + RA + 1, :],
                            in_=ht[:, :, :])

                # ---- stage B: conv1 (32 -> 32ch); fp8 DoubleRow tap pairs ----
                if 'B' in stages:
                    for s in range(H // R):
                        hin = inpool.tile([128, R + 2, WP], FP8, tag="hin")
                        nc.sync.dma_start(
                            out=hin[:, :, :],
                            in_=h0_d[b, :, s * R:s * R + R + 2, :])
                        ht = outpool.tile([128, R, WP], FP8, tag="ht")
                        nc.vector.memset(ht[:, :, :], 0.0)
                        for i in range(R // 16):
                            ps = pspool.tile([128, 4096], F32, tag="ps",
                                             name="ps")
                            for fi in range(8):
                                _conv_taps_dr(
                                    nc, ps[:, fi * 512:(fi + 1) * 512],
                                    s1t, hin, 16 * i + 2 * fi)
                            nc.scalar.activation(
                                ht[:, 16 * i:16 * i + 16, 1:W + 1],
                                ps[:, :].rearrange("p (a c) -> p a c", a=16),
                                AF.Sign)
                        nc.scalar.dma_start(
                            out=h1_d[b, :, s * R + 1:s * R + R + 1, :],
                            in_=ht[:, :, :])

                # ---- stage C: conv2 (32 -> 1ch); M zero-padded to 128 ----
                if 'C' in stages:
                    for s in range(H // R):
                        hin = inpool.tile([128, R + 2, WP], FP8, tag="hin")
                        nc.sync.dma_start(
                            out=hin[:, :, :],
                            in_=h1_d[b, :, s * R:s * R + R + 2, :])
                        ot = cpool2.tile([B, R, W], F16, tag="ot")
                        for i in range(R // 16):
                            ps = pspool.tile([128, 4096], F32, tag="ps",
                                             name="ps")
                            for fi in range(8):
                                _conv_taps_dr(
                                    nc, ps[:, fi * 512:(fi + 1) * 512],
                                    s2t, hin, 16 * i + 2 * fi)
                            nc.scalar.activation(
                                ot[:, 16 * i:16 * i + 16, :],
                                ps[0:B, :].rearrange("p (a c) -> p a c", a=16),
                                AF.Identity)
                        dst = out_d[b * B:(b + 1) * B, s * R:s * R + R, :]
                        nc.scalar.dma_start(out=dst, in_=ot[:, :, :])
    nc.compile()
    return nc


def _host_weights(w0, w1, w2):
    """Pack sign(w) into fp8 stationary matrices. tap index t = dy*3+dx."""
    w0s = np.sign(np.asarray(w0, np.float32))  # [32,1,3,3]
    w1s = np.sign(np.asarray(w1, np.float32))  # [32,32,3,3]
    w2s = np.sign(np.asarray(w2, np.float32))  # [1,32,3,3]
    s0 = np.zeros((36, 128), np.float32)
    s1 = np.zeros((128, 9, 128), np.float32)
    # s2 columns >= B are never read from PSUM (the eviction copies only
    # partitions 0:B); fill them with ones, not zeros — near-all-zero fp8
    # weight loads measure slower on this substrate.
    s2 = np.ones((128, 9, 128), np.float32)
    s2[:, :, :B] = 0.0
    for g in range(B):
        for t, (dy, dx) in enumerate(TAPS):
            # out[m=(g,co)] += s0[k=(t,g), m] * x36[k, pix]
            s0[t * B + g, g * 32:(g + 1) * 32] = w0s[:, 0, dy, dx]
            s1[g * 32:(g + 1) * 32, t, g * 32:(g + 1) * 32] = \
                w1s[:, :, dy, dx].T  # [ci, co]
            s2[g * 32:(g + 1) * 32, t, g] = w2s[0, :, dy, dx]
    return (s0.astype(ml_dtypes.float8_e4m3),
            s1.astype(ml_dtypes.float8_e4m3),
            s2.astype(ml_dtypes.float8_e4m3))


_NC_CACHE = {}


def kernel(x, w0, w1, w2):
    if "nc" not in _NC_CACHE:
        _NC_CACHE["nc"] = _build_program()
    nc = _NC_CACHE["nc"]
    s0, s1, s2 = _host_weights(w0, w1, w2)
    x = np.asarray(x, np.float32).reshape(64, H, W)
    in_maps = [
        {"x": np.ascontiguousarray(x[i * IMG_PER_CORE:(i + 1) * IMG_PER_CORE]),
         "s0": s0, "s1": s1, "s2": s2}
        for i in range(N_CORES)
    ]
    res = run_bass_kernel_spmd(nc, in_maps, list(range(N_CORES)))
    out = np.stack([np.asarray(res.results[i]["out"], np.float32)
                    for i in range(N_CORES)])
    return out.reshape(64, 1, H, W)


# revision 15
# speedup vs baseline: 1.0594x; 1.0594x over previous
"""Binary 3-layer CNN (sign activations + sign weights) on 8 NeuronCores.

Strategy: pure data parallel — 64 images -> 8 cores x 8 images.
Per core: 2 batches of 4 images; SBUF partition layout [128 = (4 img, 32 ch)].

This substrate has a large fixed cost per *instruction* (~50-100 us) that is
nearly independent of instruction size, so the design minimizes instruction
count:
 - conv0: all 9 taps packed into the contraction dim (K=36 = 9 taps x 4
   images, via 9 tap-shifted DMA loads) -> ONE matmul per PSUM fill.
 - conv1/conv2: operands in fp8e4 (values +-1/0, exact) so tap-PAIRS pack
   into one DoubleRow matmul (2 K-tiles per instruction) -> 5 matmuls per
   fill instead of 9. rhs k-tile dim is a hand-built access pattern whose
   stride is the inter-tap offset (dy*WP + dx delta). DoubleRow fp8 weight
   loads require M=128, so conv2's 4-wide weights are zero-padded to 128.
 - Matmul PSUM output is ISA-capped at one 512-f32 bank; one 8-bank pool
   tile [128, 4096] takes 8 matmul bank-slices and is evicted by ONE Sign
   activation.
All matmul operands are exactly +-1/0 -> fp32 PSUM accumulation is exact.
Layers staged through padded DRAM scratch so conv padding is baked in.
(An all-f32 variant avoids the per-matmul InstLdweights split but measures
slower: f32 matmul slots cost ~80us vs ~54us for fp8 -- fp8+DoubleRow wins.)
"""

import numpy as np
import ml_dtypes

import concourse.mybir as mybir
import concourse.tile as tile
from concourse import bacc
from concourse.bass_types import AP as RawAP
from concourse.bass_utils import run_bass_kernel_spmd

BF16 = mybir.dt.bfloat16
FP8 = mybir.dt.float8e4
F32 = mybir.dt.float32
F16 = mybir.dt.float16
AF = mybir.ActivationFunctionType
DR = mybir.MatmulPerfMode.DoubleRow

N_CORES = 8
IMG_PER_CORE = 8
B = 4          # images per partition-batch
H = W = 256
WP = 258       # padded width
HP = 258       # padded height
R = 64         # strip rows (B/C)
RA = 128       # strip rows (A)
NB = IMG_PER_CORE // B
TAPS = [(t // 3, t % 3) for t in range(9)]


def _conv_taps_dr(nc, ps_slice, s_t, hin, r):
    """9-tap conv into one 512-f32 psum bank slice: 4 DoubleRow + 1 single.
    hin is a padded fp8 tile [*, rows, WP]; r is the strip-local out row."""
    for pi, t0 in enumerate((0, 2, 4, 6)):
        dy0, dx0 = TAPS[t0]
        dy1, dx1 = TAPS[t0 + 1]
        delta = (dy1 - dy0) * WP + (dx1 - dx0)
        base = hin[:, r + dy0:r + dy0 + 2, dx0:dx0 + 256]
        rhs = RawAP(tensor=base.tensor,
                    ap=[[base.ap[0][0], 128], [delta, 2], [WP, 2], [1, 256]],
                    offset=base.offset)
        nc.tensor.matmul(ps_slice, s_t[:, t0:t0 + 2, :], rhs,
                         start=(pi == 0), stop=False, perf_mode=DR)
    nc.tensor.matmul(ps_slice, s_t[:, 8, :], hin[:, r + 2:r + 4, 2:WP],
                     start=False, stop=True)


def _build_program(stages=('0', 'A', 'B', 'C')):
    nc = bacc.Bacc("TRN2", target_bir_lowering=False, debug=False)

    x_in = nc.dram_tensor("x", [IMG_PER_CORE, H, W], F32, kind="ExternalInput")
    s0_in = nc.dram_tensor("s0", [36, 128], FP8, kind="ExternalInput")
    s1_in = nc.dram_tensor("s1", [128, 9, 128], FP8, kind="ExternalInput")
    s2_in = nc.dram_tensor("s2", [128, 9, 128], FP8, kind="ExternalInput")
    out_d = nc.dram_tensor("out", [IMG_PER_CORE, H, W], F16, kind="ExternalOutput")

    xs_d = nc.dram_tensor("xs", [IMG_PER_CORE, HP, WP], FP8)
    h0_d = nc.dram_tensor("h0", [NB, 128, HP, WP], FP8)
    h1_d = nc.dram_tensor("h1", [NB, 128, HP, WP], FP8)

    with tile.TileContext(nc) as tc:
        with (
            tc.tile_pool(name="const", bufs=1) as cpool,
            tc.tile_pool(name="xprep", bufs=2) as xpool,
            tc.tile_pool(name="a_in", bufs=1) as a_in_pool,
            tc.tile_pool(name="lay_in", bufs=2) as inpool,
            tc.tile_pool(name="lay_out", bufs=2) as outpool,
            tc.tile_pool(name="c_out", bufs=1) as cpool2,
            tc.tile_pool(name="psum", bufs=1, space="PSUM") as pspool,
        ):
            # --- constants ---
            s0t = cpool.tile([36, 128], FP8, tag="s0")
            nc.sync.dma_start(out=s0t[:, :], in_=s0_in[:, :])
            s1t = cpool.tile([128, 9, 128], FP8, tag="s1")
            nc.sync.dma_start(out=s1t[:, :, :], in_=s1_in[:, :, :])
            s2t = cpool.tile([128, 9, 128], FP8, tag="s2")
            nc.sync.dma_start(out=s2t[:, :, :], in_=s2_in[:, :, :])
            zt = cpool.tile([128, WP], FP8, tag="zt")
            nc.gpsimd.memset(zt[:, :], 0.0)
            ztw = cpool.tile([1, IMG_PER_CORE * WP], FP8, tag="ztw")
            nc.gpsimd.memset(ztw[:, :], 0.0)

            # --- pre-zero DRAM pad rows ---
            for img in range(IMG_PER_CORE):
                nc.scalar.dma_start(out=xs_d[img, 0:1, :], in_=ztw[:, 0:WP])
                nc.scalar.dma_start(out=xs_d[img, HP - 1:HP, :],
                                    in_=ztw[:, 0:WP])
            for b in range(NB):
                for hd in (h0_d, h1_d):
                    nc.scalar.dma_start(out=hd[b, :, 0, :], in_=zt[:, :])
                    nc.scalar.dma_start(out=hd[b, :, HP - 1, :], in_=zt[:, :])

            # --- stage 0: sign(x) -> padded fp8 planes in DRAM, 4 img/tile ---
            if '0' in stages:
                for b in range(NB):
                    for rb in range(H // 128):
                        xf = xpool.tile([128, B, W], F32, tag="xf")
                        src = x_in[b * B:(b + 1) * B,
                                   rb * 128:(rb + 1) * 128, :]
                        nc.sync.dma_start(out=xf[:, :, :],
                                          in_=src.transpose([1, 0, 2]))
                        xp = xpool.tile([128, B, WP], FP8, tag="xp")
                        nc.vector.memset(xp[:, :, :], 0.0)
                        nc.scalar.activation(xp[:, :, 1:W + 1], xf[:, :, :],
                                             AF.Sign)
                        dst = xs_d[b * B:(b + 1) * B,
                                   rb * 128 + 1:(rb + 1) * 128 + 1, :]
                        nc.scalar.dma_start(out=dst.transpose([1, 0, 2]),
                                            in_=xp[:, :, :])

            for b in range(NB):
                # ---- stage A: conv0 (1 -> 32ch); K=36 (9 taps x 4 img) ----
                # x36[(t,g), r, c] = xs[g, RA*s + r + dy_t, dx_t + c]
                if 'A' in stages:
                    for s in range(H // RA):
                        x36 = a_in_pool.tile([36, RA, W], FP8, tag="x36")
                        for t, (dy, dx) in enumerate(TAPS):
                            nc.sync.dma_start(
                                out=x36[B * t:B * (t + 1), :, :],
                                in_=xs_d[b * B:(b + 1) * B,
                                         s * RA + dy:s * RA + dy + RA,
                                         dx:dx + 256])
                        ht = outpool.tile([128, RA, WP], FP8, tag="hta",
                                          bufs=1)
                        nc.vector.memset(ht[:, :, :], 0.0)
                        for i in range(RA // 16):
                            ps = pspool.tile([128, 4096], F32, tag="ps",
                                             name="ps")
                            for fi in range(8):
                                r = 16 * i + 2 * fi
                                nc.tensor.matmul(
                                    ps[:, fi * 512:(fi + 1) * 512],
                                    s0t[:, :], x36[:, r:r + 2, :],
                                    start=True, stop=True)
                            nc.scalar.activation(
                                ht[:, 16 * i:16 * i + 16, 1:W + 1],
                                ps[:, :].rearrange("p (a c) -> p a c", a=16),
                                AF.Sign)
                        nc.scalar.dma_start(
                            out=h0_d[b, :, s * RA + 1:s * RA + RA + 1, :],
                            in_=ht[:, :, :])

                # ---- stage B: conv1 (32 -> 32ch); fp8 DoubleRow tap pairs ----
                if 'B' in stages:
                    for s in range(H // R):
                        hin = inpool.tile([128, R + 2, WP], FP8, tag="hin")
                        nc.sync.dma_start(
                            out=hin[:, :, :],
                            in_=h0_d[b, :, s * R:s * R + R + 2, :])
                        ht = outpool.tile([128, R, WP], FP8, tag="ht")
                        nc.vector.memset(ht[:, :, :], 0.0)
                        for i in range(R // 16):
                            ps = pspool.tile([128, 4096], F32, tag="ps",
                                             name="ps")
                            for fi in range(8):
                                _conv_taps_dr(
                                    nc, ps[:, fi * 512:(fi + 1) * 512],
                                    s1t, hin, 16 * i + 2 * fi)
                            nc.scalar.activation(
                                ht[:, 16 * i:16 * i + 16, 1:W + 1],
                                ps[:, :].rearrange("p (a c) -> p a c", a=16),
                                AF.Sign)
                        nc.scalar.dma_start(
                            out=h1_d[b, :, s * R + 1:s * R + R + 1, :],
                            in_=ht[:, :, :])

                # ---- stage C: conv2 (32 -> 1ch); M zero-padded to 128 ----
                if 'C' in stages:
                    for s in range(H // R):
                        hin = inpool.tile([128, R + 2, WP], FP8, tag="hin")
                        nc.sync.dma_start(
                            out=hin[:, :, :],
                            in_=h1_d[b, :, s * R:s * R + R + 2, :])
                        # evict all 128 psum partitions (4..127 are exact
                        # zeros from the zero-padded s2 columns); the store
                        # reads only the 4 meaningful partitions
                        ot = cpool2.tile([128, R, W], F16, tag="ot")
                        for i in range(R // 16):
                            ps = pspool.tile([128, 4096], F32, tag="ps",
                                             name="ps")
                            for fi in range(8):
                                _conv_taps_dr(
                                    nc, ps[:, fi * 512:(fi + 1) * 512],
                                    s2t, hin, 16 * i + 2 * fi)
                            nc.scalar.activation(
                                ot[:, 16 * i:16 * i + 16, :],
                                ps[:, :].rearrange("p (a c) -> p a c", a=16),
                                AF.Identity)
                        dst = out_d[b * B:(b + 1) * B, s * R:s * R + R, :]
                        nc.scalar.dma_start(out=dst, in_=ot[0:B, :, :])
    nc.compile()
    return nc


def _host_weights(w0, w1, w2):
    """Pack sign(w) into fp8 stationary matrices. tap index t = dy*3+dx."""
    w0s = np.sign(np.asarray(w0, np.float32))  # [32,1,3,3]
    w1s = np.sign(np.asarray(w1, np.float32))  # [32,32,3,3]
    w2s = np.sign(np.asarray(w2, np.float32))  # [1,32,3,3]
    s0 = np.zeros((36, 128), np.float32)
    s1 = np.zeros((128, 9, 128), np.float32)
    s2 = np.zeros((128, 9, 128), np.float32)
    for g in range(B):
        for t, (dy, dx) in enumerate(TAPS):
            # out[m=(g,co)] += s0[k=(t,g), m] * x36[k, pix]
            s0[t * B + g, g * 32:(g + 1) * 32] = w0s[:, 0, dy, dx]
            s1[g * 32:(g + 1) * 32, t, g * 32:(g + 1) * 32] = \
                w1s[:, :, dy, dx].T  # [ci, co]
            s2[g * 32:(g + 1) * 32, t, g] = w2s[0, :, dy, dx]
    return (s0.astype(ml_dtypes.float8_e4m3),
            s1.astype(ml_dtypes.float8_e4m3),
            s2.astype(ml_dtypes.float8_e4m3))


_NC_CACHE = {}


def kernel(x, w0, w1, w2):
    if "nc" not in _NC_CACHE:
        _NC_CACHE["nc"] = _build_program()
    nc = _NC_CACHE["nc"]
    s0, s1, s2 = _host_weights(w0, w1, w2)
    x = np.asarray(x, np.float32).reshape(64, H, W)
    in_maps = [
        {"x": np.ascontiguousarray(x[i * IMG_PER_CORE:(i + 1) * IMG_PER_CORE]),
         "s0": s0, "s1": s1, "s2": s2}
        for i in range(N_CORES)
    ]
    res = run_bass_kernel_spmd(nc, in_maps, list(range(N_CORES)))
    out = np.stack([np.asarray(res.results[i]["out"], np.float32)
                    for i in range(N_CORES)])
    return out.reshape(64, 1, H, W)


# revision 16
# speedup vs baseline: 1.1322x; 1.0687x over previous
"""Binary 3-layer CNN (sign activations + sign weights) on 8 NeuronCores.

Strategy: pure data parallel — 64 images -> 8 cores x 8 images.
Per core: 2 batches of 4 images; SBUF partition layout [128 = (4 img, 32 ch)].

This substrate has a large fixed cost per *instruction* (~50-100 us) that is
nearly independent of instruction size, so the design minimizes instruction
count:
 - conv0: all 9 taps packed into the contraction dim (K=36 = 9 taps x 4
   images, via 9 tap-shifted DMA loads) -> ONE matmul per PSUM fill.
 - conv1/conv2: operands in fp8e4 (values +-1/0, exact) so tap-PAIRS pack
   into one DoubleRow matmul (2 K-tiles per instruction) -> 5 matmuls per
   fill instead of 9. rhs k-tile dim is a hand-built access pattern whose
   stride is the inter-tap offset (dy*WP + dx delta). DoubleRow fp8 weight
   loads require M=128, so conv2's 4-wide weights are zero-padded to 128.
 - Matmul PSUM output is ISA-capped at one 512-f32 bank; one 8-bank pool
   tile [128, 4096] takes 8 matmul bank-slices and is evicted by ONE Sign
   activation.
All matmul operands are exactly +-1/0 -> fp32 PSUM accumulation is exact.
Layers staged through padded DRAM scratch so conv padding is baked in.
(An all-f32 variant avoids the per-matmul InstLdweights split but measures
slower: f32 matmul slots cost ~80us vs ~54us for fp8 -- fp8+DoubleRow wins.)
"""

import numpy as np
import ml_dtypes

import concourse.mybir as mybir
import concourse.tile as tile
from concourse import bacc
from concourse.bass_types import AP as RawAP
from concourse.bass_utils import run_bass_kernel_spmd

BF16 = mybir.dt.bfloat16
FP8 = mybir.dt.float8e4
F32 = mybir.dt.float32
F16 = mybir.dt.float16
AF = mybir.ActivationFunctionType
DR = mybir.MatmulPerfMode.DoubleRow

N_CORES = 8
IMG_PER_CORE = 8
B = 4          # images per partition-batch
H = W = 256
WP = 258       # padded width
HP = 258       # padded height
R = 64         # strip rows (B/C)
RA = 128       # strip rows (A)
NB = IMG_PER_CORE // B
TAPS = [(t // 3, t % 3) for t in range(9)]


def _conv_taps_dr(nc, ps_slice, s_t, hin, r):
    """9-tap conv into one 512-f32 psum bank slice: 4 DoubleRow + 1 single.
    hin is a padded fp8 tile [*, rows, WP]; r is the strip-local out row."""
    for pi, t0 in enumerate((0, 2, 4, 6)):
        dy0, dx0 = TAPS[t0]
        dy1, dx1 = TAPS[t0 + 1]
        delta = (dy1 - dy0) * WP + (dx1 - dx0)
        base = hin[:, r + dy0:r + dy0 + 2, dx0:dx0 + 256]
        rhs = RawAP(tensor=base.tensor,
                    ap=[[base.ap[0][0], 128], [delta, 2], [WP, 2], [1, 256]],
                    offset=base.offset)
        nc.tensor.matmul(ps_slice, s_t[:, t0:t0 + 2, :], rhs,
                         start=(pi == 0), stop=False, perf_mode=DR)
    nc.tensor.matmul(ps_slice, s_t[:, 8, :], hin[:, r + 2:r + 4, 2:WP],
                     start=False, stop=True)


def _build_program(stages=('0', 'A', 'B', 'C')):
    nc = bacc.Bacc("TRN2", target_bir_lowering=False, debug=False)

    x_in = nc.dram_tensor("x", [IMG_PER_CORE, H, W], F32, kind="ExternalInput")
    s0_in = nc.dram_tensor("s0", [36, 128], FP8, kind="ExternalInput")
    s1_in = nc.dram_tensor("s1", [128, 9, 128], FP8, kind="ExternalInput")
    s2_in = nc.dram_tensor("s2", [128, 9, 128], FP8, kind="ExternalInput")
    out_d = nc.dram_tensor("out", [IMG_PER_CORE, H, W], F16, kind="ExternalOutput")

    xs_d = nc.dram_tensor("xs", [IMG_PER_CORE, HP, WP], FP8)
    h0_d = nc.dram_tensor("h0", [NB, 128, HP, WP], FP8)
    h1_d = nc.dram_tensor("h1", [NB, 128, HP, WP], FP8)

    with tile.TileContext(nc) as tc:
        with (
            tc.tile_pool(name="const", bufs=1) as cpool,
            tc.tile_pool(name="xprep", bufs=2) as xpool,
            tc.tile_pool(name="a_in", bufs=1) as a_in_pool,
            tc.tile_pool(name="lay_in", bufs=2) as inpool,
            tc.tile_pool(name="lay_out", bufs=2) as outpool,
            tc.tile_pool(name="c_out", bufs=1) as cpool2,
            tc.tile_pool(name="psum", bufs=1, space="PSUM") as pspool,
        ):
            # --- constants ---
            s0t = cpool.tile([36, 128], FP8, tag="s0")
            nc.sync.dma_start(out=s0t[:, :], in_=s0_in[:, :])
            s1t = cpool.tile([128, 9, 128], FP8, tag="s1")
            nc.sync.dma_start(out=s1t[:, :, :], in_=s1_in[:, :, :])
            s2t = cpool.tile([128, 9, 128], FP8, tag="s2")
            nc.sync.dma_start(out=s2t[:, :, :], in_=s2_in[:, :, :])
            zt = cpool.tile([128, WP], FP8, tag="zt")
            nc.gpsimd.memset(zt[:, :], 0.0)
            ztw = cpool.tile([1, IMG_PER_CORE * WP], FP8, tag="ztw")
            nc.gpsimd.memset(ztw[:, :], 0.0)

            # --- pre-zero DRAM pad rows ---
            for img in range(IMG_PER_CORE):
                nc.scalar.dma_start(out=xs_d[img, 0:1, :], in_=ztw[:, 0:WP])
                nc.scalar.dma_start(out=xs_d[img, HP - 1:HP, :],
                                    in_=ztw[:, 0:WP])
            for b in range(NB):
                for hd in (h0_d, h1_d):
                    nc.scalar.dma_start(out=hd[b, :, 0, :], in_=zt[:, :])
                    nc.scalar.dma_start(out=hd[b, :, HP - 1, :], in_=zt[:, :])

            # --- stage 0: sign(x) -> padded fp8 planes in DRAM, 4 img/tile ---
            if '0' in stages:
                for b in range(NB):
                    for rb in range(H // 128):
                        xf = xpool.tile([128, B, W], F32, tag="xf")
                        src = x_in[b * B:(b + 1) * B,
                                   rb * 128:(rb + 1) * 128, :]
                        nc.sync.dma_start(out=xf[:, :, :],
                                          in_=src.transpose([1, 0, 2]))
                        xp = xpool.tile([128, B, WP], FP8, tag="xp")
                        nc.vector.memset(xp[:, :, :], 0.0)
                        nc.scalar.activation(xp[:, :, 1:W + 1], xf[:, :, :],
                                             AF.Sign)
                        dst = xs_d[b * B:(b + 1) * B,
                                   rb * 128 + 1:(rb + 1) * 128 + 1, :]
                        nc.scalar.dma_start(out=dst.transpose([1, 0, 2]),
                                            in_=xp[:, :, :])

            for b in range(NB):
                # ---- stage A: conv0 (1 -> 32ch); K=36 (9 taps x 4 img) ----
                # x36[(t,g), r, c] = xs[g, RA*s + r + dy_t, dx_t + c]
                if 'A' in stages:
                    for s in range(H // RA):
                        x36 = a_in_pool.tile([36, RA, W], FP8, tag="x36")
                        for t, (dy, dx) in enumerate(TAPS):
                            nc.sync.dma_start(
                                out=x36[B * t:B * (t + 1), :, :],
                                in_=xs_d[b * B:(b + 1) * B,
                                         s * RA + dy:s * RA + dy + RA,
                                         dx:dx + 256])
                        ht = outpool.tile([128, RA, WP], FP8, tag="hta",
                                          bufs=1)
                        nc.vector.memset(ht[:, :, :], 0.0)
                        for i in range(RA // 16):
                            ps = pspool.tile([128, 4096], F32, tag="ps",
                                             name="ps")
                            for fi in range(8):
                                r = 16 * i + 2 * fi
                                nc.tensor.matmul(
                                    ps[:, fi * 512:(fi + 1) * 512],
                                    s0t[:, :], x36[:, r:r + 2, :],
                                    start=True, stop=True)
                            nc.scalar.activation(
                                ht[:, 16 * i:16 * i + 16, 1:W + 1],
                                ps[:, :].rearrange("p (a c) -> p a c", a=16),
                                AF.Sign)
                        nc.scalar.dma_start(
                            out=h0_d[b, :, s * RA + 1:s * RA + RA + 1, :],
                            in_=ht[:, :, :])

                # ---- stage B: conv1 (32 -> 32ch); fp8 DoubleRow tap pairs ----
                if 'B' in stages:
                    for s in range(H // R):
                        hin = inpool.tile([128, R + 2, WP], FP8, tag="hin")
                        nc.sync.dma_start(
                            out=hin[:, :, :],
                            in_=h0_d[b, :, s * R:s * R + R + 2, :])
                        ht = outpool.tile([128, R, WP], FP8, tag="ht")
                        nc.vector.memset(ht[:, :, :], 0.0)
                        for i in range(R // 16):
                            ps = pspool.tile([128, 4096], F32, tag="ps",
                                             name="ps")
                            for fi in range(8):
                                _conv_taps_dr(
                                    nc, ps[:, fi * 512:(fi + 1) * 512],
                                    s1t, hin, 16 * i + 2 * fi)
                            nc.scalar.activation(
                                ht[:, 16 * i:16 * i + 16, 1:W + 1],
                                ps[:, :].rearrange("p (a c) -> p a c", a=16),
                                AF.Sign)
                        nc.scalar.dma_start(
                            out=h1_d[b, :, s * R + 1:s * R + R + 1, :],
                            in_=ht[:, :, :])

                # ---- stage C: conv2 (32 -> 1ch); M zero-padded to 128 ----
                if 'C' in stages:
                    for s in range(H // R):
                        hin = inpool.tile([128, R + 2, WP], FP8, tag="hin")
                        nc.sync.dma_start(
                            out=hin[:, :, :],
                            in_=h1_d[b, :, s * R:s * R + R + 2, :])
                        ot = cpool2.tile([B, R, W], F16, tag="ot")
                        for i in range(R // 16):
                            ps = pspool.tile([128, 4096], F32, tag="ps",
                                             name="ps")
                            for fi in range(8):
                                _conv_taps_dr(
                                    nc, ps[:, fi * 512:(fi + 1) * 512],
                                    s2t, hin, 16 * i + 2 * fi)
                            nc.scalar.activation(
                                ot[:, 16 * i:16 * i + 16, :],
                                ps[0:B, :].rearrange("p (a c) -> p a c", a=16),
                                AF.Identity)
                        dst = out_d[b * B:(b + 1) * B, s * R:s * R + R, :]
                        nc.scalar.dma_start(out=dst, in_=ot[:, :, :])
    nc.compile()
    return nc


def _host_weights(w0, w1, w2):
    """Pack sign(w) into fp8 stationary matrices. tap index t = dy*3+dx."""
    w0s = np.sign(np.asarray(w0, np.float32))  # [32,1,3,3]
    w1s = np.sign(np.asarray(w1, np.float32))  # [32,32,3,3]
    w2s = np.sign(np.asarray(w2, np.float32))  # [1,32,3,3]
    s0 = np.zeros((36, 128), np.float32)
    s1 = np.zeros((128, 9, 128), np.float32)
    s2 = np.zeros((128, 9, 128), np.float32)
    for g in range(B):
        for t, (dy, dx) in enumerate(TAPS):
            # out[m=(g,co)] += s0[k=(t,g), m] * x36[k, pix]
            s0[t * B + g, g * 32:(g + 1) * 32] = w0s[:, 0, dy, dx]
            s1[g * 32:(g + 1) * 32, t, g * 32:(g + 1) * 32] = \
                w1s[:, :, dy, dx].T  # [ci, co]
            s2[g * 32:(g + 1) * 32, t, g] = w2s[0, :, dy, dx]
    return (s0.astype(ml_dtypes.float8_e4m3),
            s1.astype(ml_dtypes.float8_e4m3),
            s2.astype(ml_dtypes.float8_e4m3))


_NC_CACHE = {}


def kernel(x, w0, w1, w2):
    if "nc" not in _NC_CACHE:
        _NC_CACHE["nc"] = _build_program()
    nc = _NC_CACHE["nc"]
    s0, s1, s2 = _host_weights(w0, w1, w2)
    x = np.asarray(x, np.float32).reshape(64, H, W)
    in_maps = [
        {"x": np.ascontiguousarray(x[i * IMG_PER_CORE:(i + 1) * IMG_PER_CORE]),
         "s0": s0, "s1": s1, "s2": s2}
        for i in range(N_CORES)
    ]
    res = run_bass_kernel_spmd(nc, in_maps, list(range(N_CORES)))
    out = np.stack([np.asarray(res.results[i]["out"], np.float32)
                    for i in range(N_CORES)])
    return out.reshape(64, 1, H, W)
